# revision 3
# baseline (speedup 1.0000x reference)
"""Trainium2 Bass kernel for ConvDownsample2d (FIR blur + 3x3/s2 conv + bias + leaky_relu*sqrt2).

Contract: kernel(**inputs) takes FULL inputs (x[16,512,64,64] f32, weight[512,512,3,3],
bias[512], fir[4,4]) and returns the FULL output [16,512,32,32] f32.

Strategy (hardcoded for this problem size):
  - Data-parallel over batch: 16 images / 8 cores = 2 images per core. No collectives.
  - Host prep: x scaled by fir[0,0] (=1/64) and cast to fp16; weights transposed to
    [cin, 3*3, cout], scaled by W_LRMUL*sqrt2, cast fp16; bias*sqrt2 as [128,4] f32.
  - Device: separable [1,3,3,1] blur on VectorE in fp16 (6 ops/chunk, all operands kept
    4B-aligned via a one-element-shifted second DMA copy of x), then the strided conv as
    accumulated 128x128x512 fp16 matmuls on TensorE (channels on partitions, 9 taps x
    4 cin-chunks into PSUM), epilogue bias+leaky_relu(0.2) on ScalarE, DMA out f32.
"""

import sys

for p in ("/opt/trn_rl_repo", "/opt/pypackages"):
    if p not in sys.path:
        sys.path.insert(0, p)

import numpy as np
from contextlib import ExitStack

from concourse import bass, bacc, mybir, tile
from concourse.bass_utils import run_bass_kernel_spmd

F16 = mybir.dt.float16
F32 = mybir.dt.float32

NCORES = 8
NPC = 2            # images per core
CIN = 512
COUT = 512
H = W = 64
OH = OW = 32
KS = 3
W_LRMUL = 1.0 / np.sqrt(CIN * COUT * KS * KS)
SQRT2 = np.sqrt(2.0)

MT = ML = 4        # top/left margin of padded SBUF tiles
SH = SW = 70       # padded tile extent (4 + 64 + 2)

_CACHE = {}


def _build():
    nc = bacc.Bacc("TRN2", target_bir_lowering=False, debug=False, enable_asserts=False)

    x_d = nc.dram_tensor("x", [NPC, CIN, H, W], F16, kind="ExternalInput")
    w_d = nc.dram_tensor("w", [CIN, 9, COUT], F16, kind="ExternalInput")
    b_d = nc.dram_tensor("b", [128, 4], F32, kind="ExternalInput")
    o_d = nc.dram_tensor("out", [NPC, COUT, OH, OW], F32, kind="ExternalOutput")

    with tile.TileContext(nc) as tc, ExitStack() as ctx:
        cpool = ctx.enter_context(tc.tile_pool(name="const", bufs=1))
        bpool = ctx.enter_context(tc.tile_pool(name="blur", bufs=1))
        opool = ctx.enter_context(tc.tile_pool(name="outp", bufs=4))
        ppool = ctx.enter_context(
            tc.tile_pool(name="psum", bufs=1, space=bass.MemorySpace.PSUM)
        )

        # --- constants ---
        w_sb = cpool.tile([128, 4, 9, COUT], F16, name="w_sb")
        for kc in range(4):
            nc.sync.dma_start(out=w_sb[:, kc], in_=w_d[kc * 128:(kc + 1) * 128])
        b_sb = cpool.tile([128, 4], F32, name="b_sb")
        nc.sync.dma_start(out=b_sb[:], in_=b_d[:])

        # --- static double-buffered blur tiles ---
        def pair(tag):
            return [
                bpool.tile([128, SH, SW], F16, tag=f"{tag}{i}", name=f"{tag}{i}")
                for i in range(2)
            ]

        xt, xst, t1t, t2t, zt, yt = (pair(t) for t in ("xt", "xs", "t1", "t2", "zt", "yt"))

        # zero guards once; every later write stays in the interior
        for tl in (*xt, *xst, *zt, *yt):
            nc.scalar.memzero(tl[:])

        AL = mybir.AluOpType

        for n in range(NPC):
            psum = [
                [
                    ppool.tile([128, 16, OW], F32, tag=f"ps{mc}{uh}", name=f"ps{mc}{uh}")
                    for uh in range(2)
                ]
                for mc in range(4)
            ]
            for kc in range(4):
                s = (n * 4 + kc) % 2
                x_, xs_, t1, t2, z, y = xt[s], xst[s], t1t[s], t2t[s], zt[s], yt[s]
                cs = x_d[n, kc * 128:(kc + 1) * 128]
                nc.sync.dma_start(out=x_[:, MT:MT + 64, ML:ML + 64], in_=cs)
                nc.sync.dma_start(out=xs_[:, MT:MT + 64, ML - 1:ML + 63], in_=cs)

                # W-blur: z[r,c] = x[c-2] + 3 x[c-1] + 3 x[c] + x[c+1]
                I = (slice(None), slice(MT, MT + 64), slice(ML, ML + 64))
                Im2 = (slice(None), slice(MT, MT + 64), slice(ML - 2, ML + 62))
                nc.vector.tensor_tensor(t1[I], x_[Im2], xs_[I], AL.add)
                nc.vector.tensor_tensor(t2[I], x_[I], xs_[Im2], AL.add)
                nc.vector.scalar_tensor_tensor(z[I], t2[I], 3.0, t1[I], AL.mult, AL.add)

                # H-blur: y[r,c] = z[r-2] + 3 z[r-1] + 3 z[r] + z[r+1]
                def rs(dr):
                    return (slice(None), slice(MT + dr, MT + dr + 64), slice(ML, ML + 64))

                nc.vector.tensor_tensor(t1[I], z[rs(-2)], z[rs(1)], AL.add)
                nc.vector.tensor_tensor(t2[I], z[rs(-1)], z[rs(0)], AL.add)
                nc.vector.scalar_tensor_tensor(y[I], t2[I], 3.0, t1[I], AL.mult, AL.add)

                # conv taps: psum[mc][uh] += w[p,q,kc,mc].T @ y[2u+p-1, 2v+q-1]
                for pq in range(9):
                    p, q = divmod(pq, 3)
                    for mc in range(4):
                        lhsT = w_sb[:, kc, pq, mc * 128:(mc + 1) * 128]
                        for uh in range(2):
                            r0 = MT - 1 + p + 32 * uh
                            c0 = ML - 1 + q
                            rhs = y[:, r0:r0 + 32:2, c0:c0 + 64:2]
                            nc.tensor.matmul(
                                psum[mc][uh][:],
                                lhsT,
                                rhs,
                                start=(kc == 0 and pq == 0),
                                stop=(kc == 3 and pq == 8),
                            )

            # epilogue: out = leaky_relu_0.2(psum + bias)   (sqrt2 folded on host)
            # ScalarE adds bias (exact f32) evacuating PSUM; VectorE does
            # leaky via max(0.2*t, t) in one scalar_tensor_tensor op.
            for mc in range(4):
                for uh in range(2):
                    tb = opool.tile([128, 16, OW], F32, tag="tb", name="tb")
                    nc.scalar.activation(
                        tb[:],
                        psum[mc][uh][:],
                        mybir.ActivationFunctionType.Identity,
                        bias=b_sb[:, mc:mc + 1],
                        scale=1.0,
                    )
                    ob = opool.tile([128, 16, OW], F32, tag="ob", name="ob")
                    nc.vector.scalar_tensor_tensor(
                        ob[:], tb[:], 0.2, tb[:], AL.mult, AL.max
                    )
                    nc.sync.dma_start(
                        out=o_d[n, mc * 128:(mc + 1) * 128, uh * 16:(uh + 1) * 16, :],
                        in_=ob[:],
                    )

    nc.compile()
    return nc


def get_nc():
    if "nc" not in _CACHE:
        _CACHE["nc"] = _build()
    return _CACHE["nc"]


def prep_inputs(x, weight, bias, fir):
    """Host-side shard + fold constants. Returns per-core input maps."""
    x = np.asarray(x, dtype=np.float32)
    weight = np.asarray(weight, dtype=np.float32)
    bias = np.asarray(bias, dtype=np.float32)
    fir = np.asarray(fir, dtype=np.float32)

    # normalized separable fir = fir[0,0] * outer([1,3,3,1],[1,3,3,1]);
    # fold fir[0,0] into x, integer taps run on device.
    scale = float(fir[0, 0])
    x_dev = (x * scale).astype(np.float16)

    # w_host[cin, p*3+q, cout] = weight[cout, cin, p, q] * W_LRMUL * sqrt2
    w_host = np.ascontiguousarray(
        (weight.transpose(1, 2, 3, 0) * np.float32(W_LRMUL * SQRT2))
        .reshape(CIN, 9, COUT)
        .astype(np.float16)
    )
    b_host = np.ascontiguousarray(
        (bias * np.float32(SQRT2)).astype(np.float32).reshape(4, 128).T
    )

    in_maps = []
    for c in range(NCORES):
        in_maps.append(
            {
                "x": np.ascontiguousarray(x_dev[c * NPC:(c + 1) * NPC]),
                "w": w_host,
                "b": b_host,
            }
        )
    return in_maps


def run(in_maps, trace=False, **kw):
    nc = get_nc()
    return run_bass_kernel_spmd(nc, in_maps, list(range(NCORES)), trace=trace, **kw)


def kernel(x, weight, bias, fir):
    res = run(prep_inputs(x, weight, bias, fir)).results
    out = np.concatenate([r["out"] for r in res], axis=0)
    return out.astype(np.float32)
